# revision 1
# baseline (speedup 1.0000x reference)
"""Trainium2 Bass kernel for ActorGCN (GCNConv(1->128) + BN + Linear + ReLU + Softmax).

Key algebraic identity used: the GCN features are rank-1 in the node state,
x[n, :] = state[n] * W + b, so the full [N, 128] message passing collapses to
two scalar segment-sums per node:
    s1[d] = dinv[d] * (sum_{e: src->d} dinv[src] * state[src] + dinv[d]*state[d])
    s0[d] = dinv[d] * (sum_{e: src->d} dinv[src]          + dinv[d])
and BatchNorm statistics collapse to scalar moments of (s1, s0).

Distribution: the 3.2M edges are sharded across 8 NeuronCores by SOURCE node
range; each core gathers u[src] = dinv[src]*state[src] for its edges (sorted by
dst), computes exact per-dst-node partial sums via chained prefix scans +
boundary gathers, and a ReduceScatter(add) combines partials so each core owns
the final sums for its node range.  BN stats use a tiny AllReduce.  All
value arithmetic (rsqrt, products, segment sums, BN, linear, softmax) runs on
device; the host only reorganizes the integer edge structure (sort/bucket/
degree counts) and pads it to fixed shapes.
"""

import os
import sys

for _p in ("/opt/trn_rl_repo", "/root/.axon_site/_ro/trn_rl_repo"):
    if os.path.isdir(_p) and _p not in sys.path:
        sys.path.append(_p)

import numpy as np

# ---------------------------------------------------------------------------
# Fixed problem geometry (hardcoded per contest rules).
N = 100000
E = 3200000
H = 128
OUT = 2
BN_EPS = 1e-5
NCORES = 8

NPP = 98                 # nodes per partition in shard layout
SH = 128 * NPP           # 12544 nodes per shard (src shard size & span size)
NTOT = NCORES * SH       # 100352 padded node space
CH = 6528                # edge-slot chunk (fits max group of real graph, 6509)
NCHUNK = 8               # chunks per Q7-core stream
NBC = SH // NCHUNK       # 896 nodes per boundary group
L_CORE = NCHUNK * CH     # edge slots per Q7-core stream
SENT = CH                # sentinel column index in prefix tile (holds carry)

TPAD = 16                # zero rows appended to the gather table
PAD_DEG = 1.0e30         # degree for padding nodes -> dinv ~ 1e-15 ~ 0

_LAST_EXEC_NS = None     # set when BASS_GCN_TRACE=1


# ---------------------------------------------------------------------------
def _host_prep(state, edge_index):
    """Build per-core integer structure + value tables. Pure layout/structure."""
    src = np.asarray(edge_index[0], dtype=np.int64)
    dst = np.asarray(edge_index[1], dtype=np.int64)
    deg = np.bincount(dst, minlength=N).astype(np.float64) + 1.0  # with self loop

    state_f = np.asarray(state, dtype=np.float32)

    deg_pad = np.full(NTOT, PAD_DEG, dtype=np.float32)
    deg_pad[:N] = deg.astype(np.float32)
    state_pad = np.zeros(NTOT, dtype=np.float32)
    state_pad[:N] = state_f

    in_maps = []
    for c in range(NCORES):
        lo, hi = c * SH, (c + 1) * SH
        sel = (src >= lo) & (src < hi)
        s_loc = (src[sel] - lo).astype(np.int32)
        d_sel = dst[sel]
        order = np.argsort(d_sel, kind="stable")
        s_loc = s_loc[order]
        d_sel = d_sel[order]

        edge_idx = np.zeros((128, L_CORE // 16), dtype=np.int16)
        bnd_idx = np.zeros((128, (NCHUNK * NBC) // 16), dtype=np.int16)

        for k in range(NCORES):
            klo, khi = k * SH, (k + 1) * SH
            a = np.searchsorted(d_sel, klo, side="left")
            b = np.searchsorted(d_sel, khi, side="left")
            sk = s_loc[a:b]
            dk = d_sel[a:b]
            # ends[i] = #edges with dst <= node (klo+i), within this stream
            ends = np.searchsorted(dk, np.arange(klo, khi), side="right")

            # group nodes into NCHUNK groups of NBC; pad each group's edges to CH
            stream = np.full(L_CORE, SH, dtype=np.int16)
            rels = np.empty(SH, dtype=np.int16)
            prev_end = 0
            for j in range(NCHUNK):
                g0, g1 = j * NBC, (j + 1) * NBC
                e0 = prev_end
                e1 = int(ends[g1 - 1])
                cnt = e1 - e0
                assert cnt <= CH, f"group overflow: {cnt} > {CH}"
                stream[j * CH : j * CH + cnt] = sk[e0:e1]
                # stream positions of this group's edges: j*CH + (local)
                ge = ends[g0:g1].astype(np.int64)
                rel = ge - 1 - e0 + j * CH  # absolute padded position of end-1
                rel_in = rel - j * CH
                r = np.where(ge - e0 > 0, rel_in, SENT).astype(np.int64)
                rels[g0:g1] = r.astype(np.int16)
                prev_end = e1

            # wrap into partitions 16k..16k+15  (position i -> part i%16, col i//16)
            edge_idx[16 * k : 16 * (k + 1), :] = stream.reshape(L_CORE // 16, 16).T
            bnd_idx[16 * k : 16 * (k + 1), :] = rels.reshape(
                (NCHUNK * NBC) // 16, 16
            ).T

        in_maps.append(
            {
                "edge_idx": edge_idx,
                "bnd_idx": bnd_idx,
                "deg_sh": deg_pad[lo:hi].copy(),
                "state_sh": state_pad[lo:hi].copy(),
            }
        )
    return in_maps


# ---------------------------------------------------------------------------
def _build_nc(DV):
    """Build the Bass program. DV=1 when gcn_b==0 (only u stream), else 2."""
    import concourse.tile as tile
    from concourse import bacc, mybir

    f32 = mybir.dt.float32
    i16 = mybir.dt.int16
    AF = mybir.ActivationFunctionType
    ALU = mybir.AluOpType

    nc = bacc.Bacc("TRN2", target_bir_lowering=False, debug=False,
                   num_devices=NCORES)

    # --- kernel I/O -------------------------------------------------------
    edge_idx = nc.dram_tensor("edge_idx", [128, L_CORE // 16], i16,
                              kind="ExternalInput").ap()
    bnd_idx = nc.dram_tensor("bnd_idx", [128, (NCHUNK * NBC) // 16], i16,
                             kind="ExternalInput").ap()
    deg_sh = nc.dram_tensor("deg_sh", [SH], f32, kind="ExternalInput").ap()
    state_sh = nc.dram_tensor("state_sh", [SH], f32, kind="ExternalInput").ap()
    gcn_W = nc.dram_tensor("gcn_W", [1, H], f32, kind="ExternalInput").ap()
    gcn_b = nc.dram_tensor("gcn_b", [H], f32, kind="ExternalInput").ap()
    bn_gamma = nc.dram_tensor("bn_gamma", [H], f32, kind="ExternalInput").ap()
    bn_beta = nc.dram_tensor("bn_beta", [H], f32, kind="ExternalInput").ap()
    lin_W = nc.dram_tensor("lin_W", [H, OUT], f32, kind="ExternalInput").ap()
    lin_b = nc.dram_tensor("lin_b", [OUT], f32, kind="ExternalInput").ap()
    out_t = nc.dram_tensor("out", [SH, OUT], f32, kind="ExternalOutput").ap()

    # --- internal DRAM ----------------------------------------------------
    tab_stage = nc.dram_tensor("tab_stage", [SH + TPAD, DV], f32)
    rs_in = nc.dram_tensor("rs_in", [NTOT, DV], f32)
    rs_out = nc.dram_tensor("rs_out", [SH, DV], f32)
    NSTAT = 2 if DV == 1 else 5
    ar_in = nc.dram_tensor("ar_in", [8], f32)
    ar_out = nc.dram_tensor("ar_out", [8], f32, addr_space="Shared")
    coef_stage = nc.dram_tensor("coef_stage", [OUT, 3], f32)

    replica = [list(range(NCORES))]

    from contextlib import ExitStack

    with tile.TileContext(nc) as tc, ExitStack() as ctx:
        persist = ctx.enter_context(tc.tile_pool(name="persist", bufs=1))
        gpool = ctx.enter_context(tc.tile_pool(name="g", bufs=2))
        ppool = ctx.enter_context(tc.tile_pool(name="p", bufs=2))
        bpool = ctx.enter_context(tc.tile_pool(name="b", bufs=2))
        spool = ctx.enter_context(tc.tile_pool(name="s", bufs=2))
        small = ctx.enter_context(tc.tile_pool(name="sm", bufs=2))
        psum = ctx.enter_context(tc.tile_pool(name="ps", bufs=2, space="PSUM"))

        # ---- 1. own-shard tables --------------------------------------
        t_deg = persist.tile([128, NPP], f32)
        nc.sync.dma_start(t_deg[:], deg_sh.rearrange("(p n) -> p n", p=128))
        t_state = persist.tile([128, NPP], f32)
        nc.sync.dma_start(t_state[:], state_sh.rearrange("(p n) -> p n", p=128))
        t_dinv = persist.tile([128, NPP], f32)
        t_rdeg = persist.tile([128, NPP], f32)
        nc.vector.reciprocal(t_rdeg[:], t_deg[:])
        nc.scalar.activation(t_dinv[:], t_rdeg[:], AF.Sqrt)
        t_uv = persist.tile([128, NPP, DV], f32)
        nc.vector.tensor_mul(t_uv[:, :, 0], t_dinv[:], t_state[:])
        if DV == 2:
            nc.vector.tensor_copy(t_uv[:, :, 1], t_dinv[:])
        nc.sync.dma_start(
            tab_stage.ap()[0:SH, :].rearrange("(p n) d -> p n d", p=128),
            t_uv[:])
        t_zpad = persist.tile([1, TPAD * DV], f32)
        nc.vector.memset(t_zpad[:], 0.0)
        nc.sync.dma_start(tab_stage.ap()[SH:, :].rearrange("n d -> (n d)"),
                          t_zpad[:])
        # replicate table across all 128 partitions
        t_table = persist.tile([128, SH + TPAD, DV], f32)
        nc.sync.dma_start(
            t_table[:],
            tab_stage.ap().rearrange("n d -> (n d)").partition_broadcast(128),
        )

        # ---- 2. edge/boundary indices to SBUF ---------------------------
        t_eidx = persist.tile([128, L_CORE // 16], i16)
        nc.sync.dma_start(t_eidx[:], edge_idx[:])
        t_bidx = persist.tile([128, (NCHUNK * NBC) // 16], i16)
        nc.sync.dma_start(t_bidx[:], bnd_idx[:])

        t_zb = persist.tile([128, 1], f32)
        nc.vector.memset(t_zb[:], 0.0)

        # carry/prev chain tiles
        prev_carry = None  # AP [128,1,DV] absolute prefix at chunk start
        prev_bval = None   # AP [128,1,DV] boundary value of previous group end

        t_zero2 = persist.tile([128, 1, DV], f32)
        nc.vector.memset(t_zero2[:], 0.0)

        # ---- 3. main loop ----------------------------------------------
        for j in range(NCHUNK):
            t_g = gpool.tile([128, CH, DV], f32, tag="gath")
            nc.gpsimd.ap_gather(
                t_g[:], t_table[:],
                t_eidx[:, j * (CH // 16):(j + 1) * (CH // 16)],
                channels=128, num_elems=SH + TPAD, d=DV, num_idxs=CH,
            )
            t_p = ppool.tile([128, CH + 1, DV], f32, tag="pref")
            # sentinel column := carry (prefix before chunk start)
            if prev_carry is None:
                nc.vector.memset(t_p[:, SENT, :], 0.0)
            else:
                nc.vector.tensor_copy(t_p[:, SENT, :], prev_carry)
            for v in range(DV):
                nc.vector.tensor_tensor_scan(
                    t_p[:, 0:CH, v], t_g[:, :, v],
                    t_zb[:].to_broadcast([128, CH]),
                    t_p[:, SENT:SENT+1, v],
                    op0=ALU.add, op1=ALU.bypass,
                )
            prev_carry = t_p[:, CH - 1, :]

            t_b = bpool.tile([128, NBC + 1, DV], f32, tag="bnd")
            if prev_bval is None:
                nc.vector.tensor_copy(t_b[:, 0, :], t_zero2[:, 0, :])
            else:
                nc.vector.tensor_copy(t_b[:, 0, :], prev_bval)
            nc.gpsimd.ap_gather(
                t_b[:, 1:, :], t_p[:],
                t_bidx[:, j * (NBC // 16):(j + 1) * (NBC // 16)],
                channels=128, num_elems=CH + 1, d=DV, num_idxs=NBC,
            )
            prev_bval = t_b[:, NBC, :]

            t_s = spool.tile([128, NBC, DV], f32, tag="sval")
            bf = t_b[:].rearrange("p n d -> p (n d)")
            nc.vector.tensor_tensor(
                t_s[:].rearrange("p n d -> p (n d)"),
                bf[:, DV:], bf[:, : NBC * DV], op=ALU.subtract,
            )
            for k in range(NCORES):
                nc.sync.dma_start(
                    rs_in.ap()[k * SH + j * NBC : k * SH + (j + 1) * NBC, :],
                    t_s[16 * k : 16 * k + 1, :, :].rearrange("p n d -> p (n d)"),
                )

        # ---- 4. ReduceScatter -------------------------------------------
        nc.gpsimd.collective_compute(
            "ReduceScatter", mybir.AluOpType.add,
            ins=[rs_in.ap()[:]], outs=[rs_out.ap()[:]],
            replica_groups=replica,
        )

        # ---- 5. tail ----------------------------------------------------
        t_agg = persist.tile([128, NPP, DV], f32)
        nc.sync.dma_start(t_agg[:], rs_out.ap().rearrange("(p n) d -> p n d", p=128))

        # s1 = dinv * (agg_u + u_own); s0 = dinv * (agg_v + v_own)
        t_s1 = persist.tile([128, NPP], f32)
        nc.vector.tensor_add(t_s1[:], t_agg[:, :, 0], t_uv[:, :, 0])
        nc.vector.tensor_mul(t_s1[:], t_s1[:], t_dinv[:])
        if DV == 2:
            t_s0 = persist.tile([128, NPP], f32)
            nc.vector.tensor_add(t_s0[:], t_agg[:, :, 1], t_uv[:, :, 1])
            nc.vector.tensor_mul(t_s0[:], t_s0[:], t_dinv[:])

        # ---- stats partials: per-partition sums -> ones-matmul -> AR ----
        t_pr = small.tile([128, NSTAT], f32)
        t_sq = small.tile([128, NPP], f32)
        nc.vector.tensor_reduce(t_pr[:, 0:1], t_s1[:], axis=mybir.AxisListType.X,
                                op=ALU.add)
        nc.vector.tensor_mul(t_sq[:], t_s1[:], t_s1[:])
        nc.vector.tensor_reduce(t_pr[:, 1:2], t_sq[:], axis=mybir.AxisListType.X,
                                op=ALU.add)
        if DV == 2:
            nc.vector.tensor_reduce(t_pr[:, 2:3], t_s0[:],
                                    axis=mybir.AxisListType.X, op=ALU.add)
            nc.vector.tensor_mul(t_sq[:], t_s0[:], t_s0[:])
            nc.vector.tensor_reduce(t_pr[:, 3:4], t_sq[:],
                                    axis=mybir.AxisListType.X, op=ALU.add)
            nc.vector.tensor_mul(t_sq[:], t_s1[:], t_s0[:])
            nc.vector.tensor_reduce(t_pr[:, 4:5], t_sq[:],
                                    axis=mybir.AxisListType.X, op=ALU.add)

        t_ones = small.tile([128, 1], f32)
        nc.vector.memset(t_ones[:], 1.0)
        ps_st = psum.tile([NSTAT, 1], f32, space="PSUM")
        nc.tensor.matmul(ps_st[:], lhsT=t_pr[:], rhs=t_ones[:], start=True,
                         stop=True)
        t_st = small.tile([NSTAT, 1], f32)
        nc.vector.tensor_copy(t_st[:], ps_st[:])
        nc.sync.dma_start(ar_in.ap()[0:NSTAT], t_st[:].rearrange("p n -> (p n)"))
        t_z8 = small.tile([1, 8 - NSTAT], f32)
        nc.vector.memset(t_z8[:], 0.0)
        nc.sync.dma_start(ar_in.ap()[NSTAT:8], t_z8[:].rearrange("p n -> (p n)"))

        nc.gpsimd.collective_compute(
            "AllReduce", mybir.AluOpType.add,
            ins=[ar_in.ap()[:]], outs=[ar_out.ap()[:]],
            replica_groups=replica,
        )

        # broadcast stats to all partitions: [128, NSTAT]
        t_stats = small.tile([128, 8], f32)
        nc.sync.dma_start(t_stats[:], ar_out.ap().partition_broadcast(128))

        # ---- coefficient computation (per-channel on partitions) --------
        t_W = small.tile([128, 1], f32)
        nc.sync.dma_start(t_W[:], gcn_W.rearrange("o h -> h o"))
        t_gam = small.tile([128, 1], f32)
        nc.sync.dma_start(t_gam[:], bn_gamma.rearrange("(h o) -> h o", o=1))
        t_bet = small.tile([128, 1], f32)
        nc.sync.dma_start(t_bet[:], bn_beta.rearrange("(h o) -> h o", o=1))
        t_lW = small.tile([128, OUT], f32)
        nc.sync.dma_start(t_lW[:], lin_W[:])

        inv_n = 1.0 / float(N)
        # moments (replicated on partitions): m1, e11 -> c11 = e11 - m1^2
        t_m = small.tile([128, 6], f32)  # m1, c11, m0, c00, c01, scratch
        nc.vector.tensor_scalar_mul(t_m[:, 0:1], t_stats[:, 0:1], inv_n)
        nc.vector.tensor_scalar_mul(t_m[:, 1:2], t_stats[:, 1:2], inv_n)
        t_tmp = small.tile([128, 1], f32)
        nc.vector.tensor_mul(t_tmp[:], t_m[:, 0:1], t_m[:, 0:1])
        nc.vector.tensor_tensor(t_m[:, 1:2], t_m[:, 1:2], t_tmp[:],
                                op=ALU.subtract)
        if DV == 2:
            nc.vector.tensor_scalar_mul(t_m[:, 2:3], t_stats[:, 2:3], inv_n)
            nc.vector.tensor_scalar_mul(t_m[:, 3:4], t_stats[:, 3:4], inv_n)
            nc.vector.tensor_mul(t_tmp[:], t_m[:, 2:3], t_m[:, 2:3])
            nc.vector.tensor_tensor(t_m[:, 3:4], t_m[:, 3:4], t_tmp[:],
                                    op=ALU.subtract)
            nc.vector.tensor_scalar_mul(t_m[:, 4:5], t_stats[:, 4:5], inv_n)
            nc.vector.tensor_mul(t_tmp[:], t_m[:, 0:1], t_m[:, 2:3])
            nc.vector.tensor_tensor(t_m[:, 4:5], t_m[:, 4:5], t_tmp[:],
                                    op=ALU.subtract)

        # var[ch] = c11*W^2 (+ 2*c01*W*b + c00*b^2)
        t_var = small.tile([128, 1], f32)
        t_w2 = small.tile([128, 1], f32)
        nc.vector.tensor_mul(t_w2[:], t_W[:], t_W[:])
        nc.vector.tensor_mul(t_var[:], t_w2[:], t_m[:, 1:2])
        if DV == 2:
            t_bv = small.tile([128, 1], f32)
            nc.sync.dma_start(t_bv[:], gcn_b.rearrange("(h o) -> h o", o=1))
            t_t2 = small.tile([128, 1], f32)
            nc.vector.tensor_mul(t_t2[:], t_W[:], t_bv[:])
            nc.vector.tensor_mul(t_t2[:], t_t2[:], t_m[:, 4:5])
            nc.vector.tensor_scalar_mul(t_t2[:], t_t2[:], 2.0)
            nc.vector.tensor_add(t_var[:], t_var[:], t_t2[:])
            nc.vector.tensor_mul(t_t2[:], t_bv[:], t_bv[:])
            nc.vector.tensor_mul(t_t2[:], t_t2[:], t_m[:, 3:4])
            nc.vector.tensor_add(t_var[:], t_var[:], t_t2[:])

        t_isd = small.tile([128, 1], f32)
        t_vpe = small.tile([128, 1], f32)
        nc.vector.tensor_scalar_add(t_vpe[:], t_var[:], BN_EPS)
        nc.vector.reciprocal(t_vpe[:], t_vpe[:])
        nc.scalar.activation(t_isd[:], t_vpe[:], AF.Sqrt)
        t_A = small.tile([128, 1], f32)
        nc.vector.tensor_mul(t_A[:], t_gam[:], t_W[:])
        nc.vector.tensor_mul(t_A[:], t_A[:], t_isd[:])
        if DV == 2:
            t_B = small.tile([128, 1], f32)
            nc.vector.tensor_mul(t_B[:], t_gam[:], t_bv[:])
            nc.vector.tensor_mul(t_B[:], t_B[:], t_isd[:])

        # a_o = sum_ch A*linW ; bw_o = sum_ch B*linW ; bet_o = sum_ch beta*linW
        NPC = 3 if DV == 2 else 2
        ps_c = psum.tile([OUT, NPC], f32, space="PSUM")
        nc.tensor.matmul(ps_c[:, 0:1], lhsT=t_lW[:], rhs=t_A[:], start=True,
                         stop=True)
        nc.tensor.matmul(ps_c[:, 1:2], lhsT=t_lW[:], rhs=t_bet[:], start=True,
                         stop=True)
        if DV == 2:
            nc.tensor.matmul(ps_c[:, 2:3], lhsT=t_lW[:], rhs=t_B[:], start=True,
                             stop=True)
        t_co = small.tile([OUT, NPC], f32)
        nc.vector.tensor_copy(t_co[:], ps_c[:])

        # c_o = -m1*a_o (- m0*bw_o) + bet_o + lin_b[o]   (on OUT partitions)
        t_lb = small.tile([OUT, 1], f32)
        nc.sync.dma_start(t_lb[:], lin_b.rearrange("(o k) -> o k", k=1))
        t_cfin = small.tile([OUT, 3], f32)  # [a, bw, c]
        nc.vector.tensor_copy(t_cfin[:, 0:1], t_co[:, 0:1])
        if DV == 2:
            nc.vector.tensor_copy(t_cfin[:, 1:2], t_co[:, 2:3])
        else:
            nc.vector.memset(t_cfin[:, 1:2], 0.0)
        t_ctmp = small.tile([OUT, 1], f32)
        nc.vector.tensor_mul(t_ctmp[:], t_co[:, 0:1], t_m[0:OUT, 0:1])
        nc.vector.tensor_tensor(t_cfin[:, 2:3], t_co[:, 1:2], t_ctmp[:],
                                op=ALU.subtract)
        if DV == 2:
            nc.vector.tensor_mul(t_ctmp[:], t_co[:, 2:3], t_m[0:OUT, 2:3])
            nc.vector.tensor_tensor(t_cfin[:, 2:3], t_cfin[:, 2:3], t_ctmp[:],
                                    op=ALU.subtract)
        nc.vector.tensor_add(t_cfin[:, 2:3], t_cfin[:, 2:3], t_lb[:])

        nc.sync.dma_start(coef_stage.ap()[:], t_cfin[:])
        t_coef = small.tile([128, OUT * 3], f32)
        nc.sync.dma_start(
            t_coef[:], coef_stage.ap().rearrange("o k -> (o k)").partition_broadcast(128)
        )
        # layout per partition: [a0, b0, c0, a1, b1, c1]

        # ---- logits + softmax -------------------------------------------
        t_l = persist.tile([128, NPP, OUT], f32)
        t_lt = small.tile([128, NPP], f32)
        for o in range(OUT):
            nc.vector.tensor_scalar_mul(t_l[:, :, o], t_s1[:],
                                        t_coef[:, 3 * o : 3 * o + 1])
            if DV == 2:
                nc.vector.tensor_scalar_mul(t_lt[:], t_s0[:],
                                            t_coef[:, 3 * o + 1 : 3 * o + 2])
                nc.vector.tensor_add(t_l[:, :, o], t_l[:, :, o], t_lt[:])
            nc.vector.tensor_scalar(t_l[:, :, o], t_l[:, :, o],
                                    t_coef[:, 3 * o + 2 : 3 * o + 3], None,
                                    op0=ALU.add)
            nc.vector.tensor_scalar_max(t_l[:, :, o], t_l[:, :, o], 0.0)

        # softmax over OUT=2: p1 = sigmoid(l1-l0), p0 = 1-p1
        t_z = small.tile([128, NPP], f32)
        nc.vector.tensor_tensor(t_z[:], t_l[:, :, 1], t_l[:, :, 0],
                                op=ALU.subtract)
        t_res = persist.tile([128, NPP, OUT], f32)
        nc.scalar.activation(t_res[:, :, 1], t_z[:], AF.Sigmoid)
        nc.vector.tensor_scalar(t_res[:, :, 0], t_res[:, :, 1], 1.0, None,
                                op0=ALU.subtract)
        nc.vector.tensor_scalar_mul(t_res[:, :, 0], t_res[:, :, 0], -1.0)

        nc.sync.dma_start(out_t.rearrange("(p n) d -> p n d", p=128), t_res[:])

    nc.compile()
    return nc


_NC_CACHE = {}


def kernel(state, edge_index, gcn_W, gcn_b, bn_gamma, bn_beta, lin_W, lin_b):
    global _LAST_EXEC_NS
    from concourse.bass_utils import run_bass_kernel_spmd

    DV = 1 if float(np.abs(np.asarray(gcn_b)).max()) == 0.0 else 2

    if DV not in _NC_CACHE:
        _NC_CACHE[DV] = _build_nc(DV)
    nc = _NC_CACHE[DV]

    in_maps = _host_prep(state, edge_index)
    shared = {
        "gcn_W": np.asarray(gcn_W, dtype=np.float32),
        "gcn_b": np.asarray(gcn_b, dtype=np.float32),
        "bn_gamma": np.asarray(bn_gamma, dtype=np.float32),
        "bn_beta": np.asarray(bn_beta, dtype=np.float32),
        "lin_W": np.asarray(lin_W, dtype=np.float32),
        "lin_b": np.asarray(lin_b, dtype=np.float32),
    }
    for m in in_maps:
        m.update(shared)

    trace = os.environ.get("BASS_GCN_TRACE", "0") == "1"
    res = run_bass_kernel_spmd(nc, in_maps, list(range(NCORES)), trace=trace)
    _LAST_EXEC_NS = res.exec_time_ns

    out = np.empty((N, OUT), dtype=np.float32)
    for c in range(NCORES):
        lo = c * SH
        hi = min(N, lo + SH)
        out[lo:hi] = res.results[c]["out"][: hi - lo]
    return out



# revision 2
# speedup vs baseline: 2.9377x; 2.9377x over previous
"""Trainium2 Bass kernel for ActorGCN (GCNConv(1->128) + BN + Linear + ReLU + Softmax).

Rank-1 identity: x[n,:] = state[n]*W + b, so message passing collapses to a
scalar segment-sum per node: s1[d] = dinv[d]*(sum_{e: s->d} dinv[s]*state[s]
+ dinv[d]*state[d]), and BN stats collapse to scalar moments.

Scheme S (scan expansion + local_scatter):
Edges are sharded across 8 NeuronCores by SOURCE shard.  Within a core the
~400K edges are laid out as 128 per-partition lanes (lane = dst % 128), each
lane's slots sorted by source.  The per-edge values u[src] are materialized
WITHOUT any gather:
  1. local_scatter #1 writes u[s] (f16, from a broadcast table) at each
     (lane, source)-run start.
  2. An affine DVE scan state = m*state + v fills values forward through runs
     -> full edge-value stream.
  3. local_scatter #2 permutes each lane's stream into dst-sorted order.
  4. A second affine scan (reset at segment starts) produces running segment
     sums; segment-end slots hold per-(lane,dst) sums.
  5. local_scatter #3 extracts segment ends into a [128 x 784] accumulator
     (dst = off*128 + lane).
A ReduceScatter(add) over the flat [100352] accumulator gives each core its
owned dst range; BN stats use a tiny AllReduce; tail math (BN/linear/softmax
collapsed to per-node scalar coefficients) is elementwise.

The host only reorganizes integer edge structure (sort/bucket/flags); all
value arithmetic runs on device.
"""

import os
import sys

for _p in ("/opt/trn_rl_repo", "/root/.axon_site/_ro/trn_rl_repo"):
    if os.path.isdir(_p) and _p not in sys.path:
        sys.path.append(_p)

import numpy as np

# ---------------------------------------------------------------------------
N = 100000
E = 3200000
H = 128
OUT = 2
BN_EPS = 1e-5
NCORES = 8

SH = 12544               # nodes per shard (= 128*98)
NPP = 98
NTOT = NCORES * SH       # 100352 padded node space
NLANE = 128
OPL = NTOT // NLANE      # 784 dst offsets per lane
PAD_DEG = 1.0e30

_LAST_EXEC_NS = None


# ---------------------------------------------------------------------------
def _host_prep_s(state, edge_index):
    """Scheme-S integer structure per core."""
    src = np.asarray(edge_index[0], dtype=np.int64)
    dst = np.asarray(edge_index[1], dtype=np.int64)
    deg = np.bincount(dst, minlength=N).astype(np.float64) + 1.0

    state_f = np.asarray(state, dtype=np.float32)
    deg_pad = np.full(NTOT, PAD_DEG, dtype=np.float32)
    deg_pad[:N] = deg.astype(np.float32)
    state_pad = np.zeros(NTOT, dtype=np.float32)
    state_pad[:N] = state_f

    pcs = []
    for c in range(NCORES):
        lo = c * SH
        sel = (src >= lo) & (src < lo + SH)
        s = (src[sel] - lo).astype(np.int32)
        d = dst[sel].astype(np.int32)
        lane = d % NLANE
        off = d // NLANE

        order = np.lexsort((s, lane))
        s, lane, off = s[order], lane[order], off[order]
        lane_cnt = np.bincount(lane, minlength=NLANE)
        lane_start = np.zeros(NLANE + 1, dtype=np.int64)
        np.cumsum(lane_cnt, out=lane_start[1:])
        slotA = np.arange(len(s), dtype=np.int64) - lane_start[lane]

        newrun = np.ones(len(s), dtype=bool)
        newrun[1:] = ~((lane[1:] == lane[:-1]) & (s[1:] == s[:-1]))

        orderB = np.lexsort((off, lane))
        laneB, offB = lane[orderB], off[orderB]
        laneB_start = np.zeros(NLANE + 1, dtype=np.int64)
        np.cumsum(np.bincount(laneB, minlength=NLANE), out=laneB_start[1:])
        slotB_B = np.arange(len(laneB), dtype=np.int64) - laneB_start[laneB]
        posB = np.empty(len(s), dtype=np.int64)
        posB[orderB] = slotB_B

        segstart = np.ones(len(laneB), dtype=bool)
        sameB = (laneB[1:] == laneB[:-1]) & (offB[1:] == offB[:-1])
        segstart[1:] = ~sameB
        segend = np.ones(len(laneB), dtype=bool)
        segend[:-1] = ~sameB

        pcs.append(dict(
            s=s, lane=lane, slotA=slotA, newrun=newrun, posB=posB,
            laneB=laneB, offB=offB, slotB_B=slotB_B,
            segstart=segstart, segend=segend,
            Wc=int(lane_cnt.max()),
        ))

    W = max(p["Wc"] for p in pcs)
    W = -(-W // 128) * 128           # stream width, multiple of 128
    W0 = W // 2
    assert W0 <= 2046, f"window {W0} exceeds local_scatter cap"

    # global lsc#1 data column ranges (uniform across cores: baked into program)
    b0 = 0
    a1 = SH
    for p in pcs:
        st = p["slotA"][p["newrun"]]
        sr = p["s"][p["newrun"]]
        in0 = st < W0
        if in0.any():
            b0 = max(b0, int(sr[in0].max()) + 1)
        if (~in0).any():
            a1 = min(a1, int(sr[~in0].min()))
    b0 = min(SH, -(-b0 // 2) * 2)
    a1 = (a1 // 2) * 2
    n1a, n1b = b0, SH - a1

    in_maps = []
    for p in pcs:
        s, lane, slotA, newrun = p["s"], p["lane"], p["slotA"], p["newrun"]
        laneB, offB, slotB_B = p["laneB"], p["offB"], p["slotB_B"]
        posB = p["posB"]

        i1a = np.full((NLANE, n1a), -1, dtype=np.int16)
        i1b = np.full((NLANE, n1b), -1, dtype=np.int16)
        st = slotA[newrun]
        sr = s[newrun]
        ln = lane[newrun]
        in0 = st < W0
        i1a[ln[in0], sr[in0]] = st[in0].astype(np.int16)
        i1b[ln[~in0], (sr[~in0] - a1)] = (st[~in0] - W0).astype(np.int16)

        mA = np.ones((NLANE, W), dtype=np.float16)
        mA[ln, st] = 0.0

        iB0 = np.full((NLANE, W), -1, dtype=np.int16)
        iB1 = np.full((NLANE, W), -1, dtype=np.int16)
        pb_lo = posB < W0
        iB0[lane[pb_lo], slotA[pb_lo]] = posB[pb_lo].astype(np.int16)
        iB1[lane[~pb_lo], slotA[~pb_lo]] = (posB[~pb_lo] - W0).astype(np.int16)

        mB = np.ones((NLANE, W), dtype=np.float16)
        ssl = p["segstart"]
        mB[laneB[ssl], slotB_B[ssl]] = 0.0

        iC = np.full((NLANE, W), -1, dtype=np.int16)
        sel_ = p["segend"]
        iC[laneB[sel_], slotB_B[sel_]] = offB[sel_].astype(np.int16)

        in_maps.append(dict(i1a=i1a, i1b=i1b, mA=mA, iB0=iB0, iB1=iB1,
                            mB=mB, iC=iC))

    for c in range(NCORES):
        lo = c * SH
        in_maps[c]["deg_sh"] = deg_pad[lo:lo + SH].copy()
        in_maps[c]["state_sh"] = state_pad[lo:lo + SH].copy()

    return in_maps, W, b0, a1


# ---------------------------------------------------------------------------
def _build_nc_s(W, b0, a1):
    import concourse.tile as tile
    from concourse import bacc, mybir

    f32 = mybir.dt.float32
    f16 = mybir.dt.float16
    i16 = mybir.dt.int16
    AF = mybir.ActivationFunctionType
    ALU = mybir.AluOpType

    W0 = W // 2
    n1a, n1b = b0, SH - a1

    nc = bacc.Bacc("TRN2", target_bir_lowering=False, debug=False,
                   num_devices=NCORES)

    deg_sh = nc.dram_tensor("deg_sh", [SH], f32, kind="ExternalInput").ap()
    state_sh = nc.dram_tensor("state_sh", [SH], f32, kind="ExternalInput").ap()
    i1a_t = nc.dram_tensor("i1a", [NLANE, n1a], i16, kind="ExternalInput").ap()
    i1b_t = nc.dram_tensor("i1b", [NLANE, n1b], i16, kind="ExternalInput").ap()
    mA_t = nc.dram_tensor("mA", [NLANE, W], f16, kind="ExternalInput").ap()
    iB0_t = nc.dram_tensor("iB0", [NLANE, W], i16, kind="ExternalInput").ap()
    iB1_t = nc.dram_tensor("iB1", [NLANE, W], i16, kind="ExternalInput").ap()
    mB_t = nc.dram_tensor("mB", [NLANE, W], f16, kind="ExternalInput").ap()
    iC_t = nc.dram_tensor("iC", [NLANE, W], i16, kind="ExternalInput").ap()
    gcn_W = nc.dram_tensor("gcn_W", [1, H], f32, kind="ExternalInput").ap()
    bn_gamma = nc.dram_tensor("bn_gamma", [H], f32, kind="ExternalInput").ap()
    bn_beta = nc.dram_tensor("bn_beta", [H], f32, kind="ExternalInput").ap()
    lin_W = nc.dram_tensor("lin_W", [H, OUT], f32, kind="ExternalInput").ap()
    lin_b = nc.dram_tensor("lin_b", [OUT], f32, kind="ExternalInput").ap()
    out_t = nc.dram_tensor("out", [SH, OUT], f32, kind="ExternalOutput").ap()

    u_stage = nc.dram_tensor("u_stage", [SH], f16)
    rs_in = nc.dram_tensor("rs_in", [NTOT], f32)
    rs_out = nc.dram_tensor("rs_out", [SH], f32)
    ar_in = nc.dram_tensor("ar_in", [8], f32)
    ar_out = nc.dram_tensor("ar_out", [8], f32, addr_space="Shared")

    replica = [list(range(NCORES))]

    from contextlib import ExitStack

    with tile.TileContext(nc) as tc, ExitStack() as ctx:
        pp = ctx.enter_context(tc.tile_pool(name="pp", bufs=1))
        small = ctx.enter_context(tc.tile_pool(name="sm", bufs=2))
        psum = ctx.enter_context(tc.tile_pool(name="ps", bufs=2, space="PSUM"))

        # ---- u table: u = state/sqrt(deg), natural (p n) layout ----------
        t_deg = pp.tile([128, NPP], f32)
        nc.sync.dma_start(t_deg[:], deg_sh.rearrange("(p n) -> p n", p=128))
        t_state = pp.tile([128, NPP], f32)
        nc.sync.dma_start(t_state[:], state_sh.rearrange("(p n) -> p n", p=128))
        t_tmp = pp.tile([128, NPP], f32)
        t_dinv = pp.tile([128, NPP], f32)
        nc.vector.reciprocal(t_tmp[:], t_deg[:])
        nc.scalar.activation(t_dinv[:], t_tmp[:], AF.Sqrt)
        t_u32 = pp.tile([128, NPP], f32)
        nc.vector.tensor_mul(t_u32[:], t_dinv[:], t_state[:])
        t_u16 = pp.tile([128, NPP], f16)
        nc.vector.tensor_copy(t_u16[:], t_u32[:])
        nc.sync.dma_start(u_stage.ap().rearrange("(p n) -> p n", p=128),
                          t_u16[:])
        t_utab = pp.tile([128, SH], f16)
        nc.sync.dma_start(t_utab[:], u_stage.ap().partition_broadcast(128))

        # ---- tail-layout tables (dst = n*128 + p) ------------------------
        t_deg2 = pp.tile([128, NPP], f32)
        nc.sync.dma_start(t_deg2[:], deg_sh.rearrange("(n p) -> p n", p=128))
        t_state2 = pp.tile([128, NPP], f32)
        nc.sync.dma_start(t_state2[:], state_sh.rearrange("(n p) -> p n", p=128))
        t_dinv2 = pp.tile([128, NPP], f32)
        nc.vector.reciprocal(t_tmp[:], t_deg2[:])
        nc.scalar.activation(t_dinv2[:], t_tmp[:], AF.Sqrt)
        t_uown = pp.tile([128, NPP], f32)
        nc.vector.tensor_mul(t_uown[:], t_dinv2[:], t_state2[:])

        # ---- structure tensors ------------------------------------------
        t_i1a = pp.tile([128, n1a], i16)
        nc.sync.dma_start(t_i1a[:], i1a_t[:])
        t_i1b = pp.tile([128, n1b], i16)
        nc.sync.dma_start(t_i1b[:], i1b_t[:])
        t_mA = pp.tile([128, W], f16)
        nc.sync.dma_start(t_mA[:], mA_t[:])
        t_iB0 = pp.tile([128, W], i16)
        nc.sync.dma_start(t_iB0[:], iB0_t[:])
        t_iB1 = pp.tile([128, W], i16)
        nc.sync.dma_start(t_iB1[:], iB1_t[:])
        t_mB = pp.tile([128, W], f16)
        nc.sync.dma_start(t_mB[:], mB_t[:])
        t_iC = pp.tile([128, W], i16)
        nc.sync.dma_start(t_iC[:], iC_t[:])

        # ---- 1. scatter run-start values --------------------------------
        t_v0 = pp.tile([128, W], f16)
        nc.gpsimd.local_scatter(
            t_v0[:, 0:W0], t_utab[:, 0:b0], t_i1a[:],
            channels=128, num_elems=W0, num_idxs=n1a)
        nc.gpsimd.local_scatter(
            t_v0[:, W0:W], t_utab[:, a1:SH], t_i1b[:],
            channels=128, num_elems=W - W0, num_idxs=n1b)

        # ---- 2. fill-forward scan: state = mA*state + v0 ----------------
        t_w16 = pp.tile([128, W], f16)
        nc.vector.tensor_tensor_scan(
            t_w16[:], t_mA[:], t_v0[:], 0.0,
            op0=ALU.mult, op1=ALU.add)

        # ---- 3. permute to dst-sorted order -----------------------------
        t_z = pp.tile([128, W], f16)
        nc.gpsimd.local_scatter(
            t_z[:, 0:W0], t_w16[:], t_iB0[:],
            channels=128, num_elems=W0, num_idxs=W)
        nc.gpsimd.local_scatter(
            t_z[:, W0:W], t_w16[:], t_iB1[:],
            channels=128, num_elems=W - W0, num_idxs=W)

        # ---- 4. segment-sum scan: state = mB*state + z ------------------
        t_seg = pp.tile([128, W], f16)
        nc.vector.tensor_tensor_scan(
            t_seg[:], t_mB[:], t_z[:], 0.0,
            op0=ALU.mult, op1=ALU.add)

        # ---- 5. extract segment ends into accumulator -------------------
        t_acc16 = pp.tile([128, OPL], f16)
        nc.gpsimd.local_scatter(
            t_acc16[:], t_seg[:], t_iC[:],
            channels=128, num_elems=OPL, num_idxs=W)
        t_acc32 = pp.tile([128, OPL], f32)
        nc.vector.tensor_copy(t_acc32[:], t_acc16[:])
        nc.sync.dma_start(rs_in.ap().rearrange("(o p) -> p o", p=128),
                          t_acc32[:])

        # ---- 6. ReduceScatter -------------------------------------------
        nc.gpsimd.collective_compute(
            "ReduceScatter", mybir.AluOpType.add,
            ins=[rs_in.ap()[:]], outs=[rs_out.ap()[:]],
            replica_groups=replica,
        )

        # ---- 7. tail -----------------------------------------------------
        t_agg = pp.tile([128, NPP], f32)
        nc.sync.dma_start(t_agg[:], rs_out.ap().rearrange("(n p) -> p n", p=128))
        t_s1 = pp.tile([128, NPP], f32)
        nc.vector.tensor_add(t_s1[:], t_agg[:], t_uown[:])
        nc.vector.tensor_mul(t_s1[:], t_s1[:], t_dinv2[:])

        # stats partials -> ones-matmul -> AllReduce
        NSTAT = 2
        t_pr = small.tile([128, NSTAT], f32)
        t_sq = small.tile([128, NPP], f32)
        nc.vector.tensor_reduce(t_pr[:, 0:1], t_s1[:], axis=mybir.AxisListType.X,
                                op=ALU.add)
        nc.vector.tensor_mul(t_sq[:], t_s1[:], t_s1[:])
        nc.vector.tensor_reduce(t_pr[:, 1:2], t_sq[:], axis=mybir.AxisListType.X,
                                op=ALU.add)
        t_ones = small.tile([128, 1], f32)
        nc.vector.memset(t_ones[:], 1.0)
        ps_st = psum.tile([NSTAT, 1], f32, space="PSUM")
        nc.tensor.matmul(ps_st[:], lhsT=t_pr[:], rhs=t_ones[:], start=True,
                         stop=True)
        t_st = small.tile([NSTAT, 1], f32)
        nc.vector.tensor_copy(t_st[:], ps_st[:])
        nc.sync.dma_start(ar_in.ap()[0:NSTAT], t_st[:].rearrange("p n -> (p n)"))
        t_z8 = small.tile([1, 8 - NSTAT], f32)
        nc.vector.memset(t_z8[:], 0.0)
        nc.sync.dma_start(ar_in.ap()[NSTAT:8], t_z8[:].rearrange("p n -> (p n)"))

        nc.gpsimd.collective_compute(
            "AllReduce", mybir.AluOpType.add,
            ins=[ar_in.ap()[:]], outs=[ar_out.ap()[:]],
            replica_groups=replica,
        )
        t_stats = small.tile([128, 8], f32)
        nc.sync.dma_start(t_stats[:], ar_out.ap().partition_broadcast(128))

        # per-channel coefficients
        t_W = small.tile([128, 1], f32)
        nc.sync.dma_start(t_W[:], gcn_W.rearrange("o h -> h o"))
        t_gam = small.tile([128, 1], f32)
        nc.sync.dma_start(t_gam[:], bn_gamma.rearrange("(h o) -> h o", o=1))
        t_bet = small.tile([128, 1], f32)
        nc.sync.dma_start(t_bet[:], bn_beta.rearrange("(h o) -> h o", o=1))
        t_lW = small.tile([128, OUT], f32)
        nc.sync.dma_start(t_lW[:], lin_W[:])

        inv_n = 1.0 / float(N)
        t_m = small.tile([128, 2], f32)  # m1, c11
        nc.vector.tensor_scalar_mul(t_m[:, 0:1], t_stats[:, 0:1], inv_n)
        nc.vector.tensor_scalar_mul(t_m[:, 1:2], t_stats[:, 1:2], inv_n)
        t_t1 = small.tile([128, 1], f32)
        nc.vector.tensor_mul(t_t1[:], t_m[:, 0:1], t_m[:, 0:1])
        nc.vector.tensor_tensor(t_m[:, 1:2], t_m[:, 1:2], t_t1[:],
                                op=ALU.subtract)

        t_var = small.tile([128, 1], f32)
        t_w2 = small.tile([128, 1], f32)
        nc.vector.tensor_mul(t_w2[:], t_W[:], t_W[:])
        nc.vector.tensor_mul(t_var[:], t_w2[:], t_m[:, 1:2])
        t_isd = small.tile([128, 1], f32)
        t_vpe = small.tile([128, 1], f32)
        nc.vector.tensor_scalar_add(t_vpe[:], t_var[:], BN_EPS)
        nc.vector.reciprocal(t_vpe[:], t_vpe[:])
        nc.scalar.activation(t_isd[:], t_vpe[:], AF.Sqrt)
        t_A = small.tile([128, 1], f32)
        nc.vector.tensor_mul(t_A[:], t_gam[:], t_W[:])
        nc.vector.tensor_mul(t_A[:], t_A[:], t_isd[:])

        # a_o = sum_ch A*linW ; bet_o = sum_ch beta*linW
        ps_c = psum.tile([OUT, 2], f32, space="PSUM")
        nc.tensor.matmul(ps_c[:, 0:1], lhsT=t_lW[:], rhs=t_A[:], start=True,
                         stop=True)
        nc.tensor.matmul(ps_c[:, 1:2], lhsT=t_lW[:], rhs=t_bet[:], start=True,
                         stop=True)
        t_co = small.tile([OUT, 2], f32)
        nc.vector.tensor_copy(t_co[:], ps_c[:])

        # c_o = -m1*a_o + bet_o + lin_b[o]
        coef_stage = nc.dram_tensor("coef_stage", [OUT, 2], f32)
        t_lb = small.tile([OUT, 1], f32)
        nc.sync.dma_start(t_lb[:], lin_b.rearrange("(o k) -> o k", k=1))
        t_cfin = small.tile([OUT, 2], f32)  # [a, c]
        nc.vector.tensor_copy(t_cfin[:, 0:1], t_co[:, 0:1])
        t_ct = small.tile([OUT, 1], f32)
        nc.vector.tensor_mul(t_ct[:], t_co[:, 0:1], t_m[0:OUT, 0:1])
        nc.vector.tensor_tensor(t_cfin[:, 1:2], t_co[:, 1:2], t_ct[:],
                                op=ALU.subtract)
        nc.vector.tensor_add(t_cfin[:, 1:2], t_cfin[:, 1:2], t_lb[:])

        nc.sync.dma_start(coef_stage.ap()[:], t_cfin[:])
        t_coef = small.tile([128, OUT * 2], f32)
        nc.sync.dma_start(
            t_coef[:],
            coef_stage.ap().rearrange("o k -> (o k)").partition_broadcast(128))
        # per partition: [a0, c0, a1, c1]

        # logits + softmax (softmax over 2 = sigmoid of diff)
        t_l = pp.tile([128, NPP, OUT], f32)
        for o in range(OUT):
            nc.vector.tensor_scalar_mul(t_l[:, :, o], t_s1[:],
                                        t_coef[:, 2 * o: 2 * o + 1])
            nc.vector.tensor_scalar(t_l[:, :, o], t_l[:, :, o],
                                    t_coef[:, 2 * o + 1: 2 * o + 2], None,
                                    op0=ALU.add)
            nc.vector.tensor_scalar_max(t_l[:, :, o], t_l[:, :, o], 0.0)

        t_zd = small.tile([128, NPP], f32)
        nc.vector.tensor_tensor(t_zd[:], t_l[:, :, 1], t_l[:, :, 0],
                                op=ALU.subtract)
        t_res = pp.tile([128, NPP, OUT], f32)
        nc.scalar.activation(t_res[:, :, 1], t_zd[:], AF.Sigmoid)
        nc.vector.tensor_scalar(t_res[:, :, 0], t_res[:, :, 1], 1.0, None,
                                op0=ALU.subtract)
        nc.vector.tensor_scalar_mul(t_res[:, :, 0], t_res[:, :, 0], -1.0)

        nc.sync.dma_start(out_t.rearrange("(n p) d -> p n d", p=128), t_res[:])

    nc.compile()
    return nc


_NC_CACHE = {}


def _kernel_s(state, edge_index, gcn_W, gcn_b, bn_gamma, bn_beta, lin_W, lin_b):
    global _LAST_EXEC_NS
    from concourse.bass_utils import run_bass_kernel_spmd

    in_maps, W, b0, a1 = _host_prep_s(state, edge_index)
    key = ("s", W, b0, a1)
    if key not in _NC_CACHE:
        _NC_CACHE[key] = _build_nc_s(W, b0, a1)
    nc = _NC_CACHE[key]

    shared = {
        "gcn_W": np.asarray(gcn_W, dtype=np.float32),
        "bn_gamma": np.asarray(bn_gamma, dtype=np.float32),
        "bn_beta": np.asarray(bn_beta, dtype=np.float32),
        "lin_W": np.asarray(lin_W, dtype=np.float32),
        "lin_b": np.asarray(lin_b, dtype=np.float32),
    }
    for m in in_maps:
        m.update(shared)

    trace = os.environ.get("BASS_GCN_TRACE", "0") == "1"
    res = run_bass_kernel_spmd(nc, in_maps, list(range(NCORES)), trace=trace)
    _LAST_EXEC_NS = res.exec_time_ns

    out = np.empty((N, OUT), dtype=np.float32)
    for c in range(NCORES):
        lo = c * SH
        hi = min(N, lo + SH)
        out[lo:hi] = res.results[c]["out"][: hi - lo]
    return out


def kernel(state, edge_index, gcn_W, gcn_b, bn_gamma, bn_beta, lin_W, lin_b):
    global _LAST_EXEC_NS
    if float(np.abs(np.asarray(gcn_b)).max()) == 0.0:
        return _kernel_s(state, edge_index, gcn_W, gcn_b, bn_gamma, bn_beta,
                         lin_W, lin_b)
    # fallback: original implementation (handles gcn_b != 0)
    import kernel_v1_backup as _v1
    out = _v1.kernel(state, edge_index, gcn_W, gcn_b, bn_gamma, bn_beta,
                     lin_W, lin_b)
    _LAST_EXEC_NS = _v1._LAST_EXEC_NS
    return out


# revision 9
# speedup vs baseline: 9.2495x; 3.1485x over previous
"""Trainium2 Bass kernel for ActorGCN (GCNConv(1->128) + BN + Linear + ReLU + Softmax).

Rank-1 identity: x[n,:] = state[n]*W + b, so message passing collapses to a
scalar segment-sum per node: s1[d] = dinv[d]*(sum_{e: s->d} dinv[s]*state[s]
+ dinv[d]*state[d]), and BN stats collapse to scalar moments.

Scheme S (scan expansion + local_scatter):
Edges are sharded across 8 NeuronCores by SOURCE shard.  Within a core the
~400K edges are laid out as 128 per-partition lanes (lane = dst % 128), each
lane's slots sorted by source.  The per-edge values u[src] are materialized
WITHOUT any gather:
  1. local_scatter #1 writes u[s] (f16, from a broadcast table) at each
     (lane, source)-run start.
  2. An affine DVE scan state = m*state + v fills values forward through runs
     -> full edge-value stream.
  3. local_scatter #2 permutes each lane's stream into dst-sorted order.
  4. A second affine scan (reset at segment starts) produces running segment
     sums; segment-end slots hold per-(lane,dst) sums.
  5. local_scatter #3 extracts segment ends into a [128 x 784] accumulator
     (dst = off*128 + lane).
A ReduceScatter(add) over the flat [100352] accumulator gives each core its
owned dst range; BN stats use a tiny AllReduce; tail math (BN/linear/softmax
collapsed to per-node scalar coefficients) is elementwise.

The host only reorganizes integer edge structure (sort/bucket/flags); all
value arithmetic runs on device.
"""

import os
import sys

for _p in ("/opt/trn_rl_repo", "/root/.axon_site/_ro/trn_rl_repo"):
    if os.path.isdir(_p) and _p not in sys.path:
        sys.path.append(_p)

import numpy as np

# ---------------------------------------------------------------------------
N = 100000
E = 3200000
H = 128
OUT = 2
BN_EPS = 1e-5
NCORES = 8

SH = 12544               # nodes per shard (= 128*98)
NPP = 98
NTOT = NCORES * SH       # 100352 padded node space
NLANE = 128
OPL = NTOT // NLANE      # 784 dst offsets per lane
PAD_DEG = 1.0e30

_LAST_EXEC_NS = None


# ---------------------------------------------------------------------------
def _host_prep_s(state, edge_index):
    """Scheme-S integer structure per core."""
    src = np.asarray(edge_index[0], dtype=np.int64)
    dst = np.asarray(edge_index[1], dtype=np.int64)
    deg = np.bincount(dst, minlength=N).astype(np.float64) + 1.0

    state_f = np.asarray(state, dtype=np.float32)
    deg_pad = np.full(NTOT, PAD_DEG, dtype=np.float32)
    deg_pad[:N] = deg.astype(np.float32)
    state_pad = np.zeros(NTOT, dtype=np.float32)
    state_pad[:N] = state_f

    pcs = []
    for c in range(NCORES):
        lo = c * SH
        sel = (src >= lo) & (src < lo + SH)
        s = (src[sel] - lo).astype(np.int32)
        d = dst[sel].astype(np.int32)
        lane = d % NLANE
        off = d // NLANE

        order = np.lexsort((s, lane))
        s, lane, off = s[order], lane[order], off[order]
        lane_cnt = np.bincount(lane, minlength=NLANE)
        lane_start = np.zeros(NLANE + 1, dtype=np.int64)
        np.cumsum(lane_cnt, out=lane_start[1:])
        slotA = np.arange(len(s), dtype=np.int64) - lane_start[lane]

        newrun = np.ones(len(s), dtype=bool)
        newrun[1:] = ~((lane[1:] == lane[:-1]) & (s[1:] == s[:-1]))

        orderB = np.lexsort((off, lane))
        laneB, offB = lane[orderB], off[orderB]
        laneB_start = np.zeros(NLANE + 1, dtype=np.int64)
        np.cumsum(np.bincount(laneB, minlength=NLANE), out=laneB_start[1:])
        slotB_B = np.arange(len(laneB), dtype=np.int64) - laneB_start[laneB]
        posB = np.empty(len(s), dtype=np.int64)
        posB[orderB] = slotB_B

        segstart = np.ones(len(laneB), dtype=bool)
        sameB = (laneB[1:] == laneB[:-1]) & (offB[1:] == offB[:-1])
        segstart[1:] = ~sameB
        segend = np.ones(len(laneB), dtype=bool)
        segend[:-1] = ~sameB

        pcs.append(dict(
            s=s, lane=lane, slotA=slotA, newrun=newrun, posB=posB,
            laneB=laneB, offB=offB, slotB_B=slotB_B,
            segstart=segstart, segend=segend,
            Wc=int(lane_cnt.max()),
        ))

    W = max(p["Wc"] for p in pcs)
    W = -(-W // 128) * 128           # stream width, multiple of 128
    W0 = W // 2
    assert W0 <= 2046, f"window {W0} exceeds local_scatter cap"

    # global lsc#1 data column ranges (uniform across cores: baked into program)
    b0 = 0
    a1 = SH
    for p in pcs:
        st = p["slotA"][p["newrun"]]
        sr = p["s"][p["newrun"]]
        in0 = st < W0
        if in0.any():
            b0 = max(b0, int(sr[in0].max()) + 1)
        if (~in0).any():
            a1 = min(a1, int(sr[~in0].min()))
    b0 = min(SH, -(-b0 // 2) * 2)
    a1 = (a1 // 2) * 2
    n1a, n1b = b0, SH - a1

    in_maps = []
    for p in pcs:
        s, lane, slotA, newrun = p["s"], p["lane"], p["slotA"], p["newrun"]
        laneB, offB, slotB_B = p["laneB"], p["offB"], p["slotB_B"]
        posB = p["posB"]

        i1a = np.full((NLANE, n1a), -1, dtype=np.int16)
        i1b = np.full((NLANE, n1b), -1, dtype=np.int16)
        st = slotA[newrun]
        sr = s[newrun]
        ln = lane[newrun]
        in0 = st < W0
        i1a[ln[in0], sr[in0]] = st[in0].astype(np.int16)
        i1b[ln[~in0], (sr[~in0] - a1)] = (st[~in0] - W0).astype(np.int16)

        mA = np.ones((NLANE, W), dtype=np.float16)
        mA[ln, st] = 0.0

        iB0 = np.full((NLANE, W), -1, dtype=np.int16)
        iB1 = np.full((NLANE, W), -1, dtype=np.int16)
        pb_lo = posB < W0
        iB0[lane[pb_lo], slotA[pb_lo]] = posB[pb_lo].astype(np.int16)
        iB1[lane[~pb_lo], slotA[~pb_lo]] = (posB[~pb_lo] - W0).astype(np.int16)

        mB = np.ones((NLANE, W), dtype=np.float16)
        ssl = p["segstart"]
        mB[laneB[ssl], slotB_B[ssl]] = 0.0

        iC = np.full((NLANE, W), -1, dtype=np.int16)
        sel_ = p["segend"]
        iC[laneB[sel_], slotB_B[sel_]] = offB[sel_].astype(np.int16)

        in_maps.append(dict(i1a=i1a, i1b=i1b, mA=mA, iB0=iB0, iB1=iB1,
                            mB=mB, iC=iC))

    for c in range(NCORES):
        lo = c * SH
        in_maps[c]["deg_sh"] = deg_pad[lo:lo + SH].copy()
        in_maps[c]["state_sh"] = state_pad[lo:lo + SH].copy()
        # owned dsts after lane-major ReduceScatter: row i of rs_out is
        # dst o*128 + p with p = 16c + i//OPL, o = i%OPL
        i_ = np.arange(SH, dtype=np.int64)
        dstg = (i_ % OPL) * NLANE + 16 * c + i_ // OPL
        in_maps[c]["deg_own"] = deg_pad[dstg].copy()
        in_maps[c]["state_own"] = state_pad[dstg].copy()

    return in_maps, W, b0, a1


# ---------------------------------------------------------------------------
def _build_nc_s(W, b0, a1):
    import concourse.tile as tile
    from concourse import bacc, mybir

    f32 = mybir.dt.float32
    f16 = mybir.dt.float16
    i16 = mybir.dt.int16
    AF = mybir.ActivationFunctionType
    ALU = mybir.AluOpType

    W0 = W // 2
    n1a, n1b = b0, SH - a1

    nc = bacc.Bacc("TRN2", target_bir_lowering=False, debug=False,
                   num_devices=NCORES)

    deg_sh = nc.dram_tensor("deg_sh", [SH], f32, kind="ExternalInput").ap()
    state_sh = nc.dram_tensor("state_sh", [SH], f32, kind="ExternalInput").ap()
    deg_own = nc.dram_tensor("deg_own", [SH], f32, kind="ExternalInput").ap()
    state_own = nc.dram_tensor("state_own", [SH], f32, kind="ExternalInput").ap()
    i1a_t = nc.dram_tensor("i1a", [NLANE, n1a], i16, kind="ExternalInput").ap()
    i1b_t = nc.dram_tensor("i1b", [NLANE, n1b], i16, kind="ExternalInput").ap()
    mA_t = nc.dram_tensor("mA", [NLANE, W], f16, kind="ExternalInput").ap()
    iB0_t = nc.dram_tensor("iB0", [NLANE, W], i16, kind="ExternalInput").ap()
    iB1_t = nc.dram_tensor("iB1", [NLANE, W], i16, kind="ExternalInput").ap()
    mB_t = nc.dram_tensor("mB", [NLANE, W], f16, kind="ExternalInput").ap()
    iC_t = nc.dram_tensor("iC", [NLANE, W], i16, kind="ExternalInput").ap()
    gcn_W = nc.dram_tensor("gcn_W", [1, H], f32, kind="ExternalInput").ap()
    bn_gamma = nc.dram_tensor("bn_gamma", [H], f32, kind="ExternalInput").ap()
    bn_beta = nc.dram_tensor("bn_beta", [H], f32, kind="ExternalInput").ap()
    lin_W = nc.dram_tensor("lin_W", [H, OUT], f32, kind="ExternalInput").ap()
    lin_b = nc.dram_tensor("lin_b", [OUT], f32, kind="ExternalInput").ap()
    out_t = nc.dram_tensor("out", [SH, OUT], f32, kind="ExternalOutput").ap()

    u_stage = nc.dram_tensor("u_stage", [SH], f16)
    rs_in = nc.dram_tensor("rs_in", [NTOT], f32)
    rs_out = nc.dram_tensor("rs_out", [SH], f32)
    ar_in = nc.dram_tensor("ar_in", [8], f32)
    ar_out = nc.dram_tensor("ar_out", [8], f32, addr_space="Shared")

    replica = [list(range(NCORES))]

    from contextlib import ExitStack

    with tile.TileContext(nc) as tc, ExitStack() as ctx:
        pp = ctx.enter_context(tc.tile_pool(name="pp", bufs=1))
        small = ctx.enter_context(tc.tile_pool(name="sm", bufs=2))
        psum = ctx.enter_context(tc.tile_pool(name="ps", bufs=2, space="PSUM"))

        # ---- u table: u = state/sqrt(deg), natural (p n) layout ----------
        t_deg = pp.tile([128, NPP], f32)
        nc.sync.dma_start(t_deg[:], deg_sh.rearrange("(p n) -> p n", p=128))
        t_state = pp.tile([128, NPP], f32)
        nc.sync.dma_start(t_state[:], state_sh.rearrange("(p n) -> p n", p=128))
        t_tmp = pp.tile([128, NPP], f32)
        t_dinv = pp.tile([128, NPP], f32)
        nc.vector.reciprocal(t_tmp[:], t_deg[:])
        nc.scalar.activation(t_dinv[:], t_tmp[:], AF.Sqrt)
        t_u32 = pp.tile([128, NPP], f32)
        nc.vector.tensor_mul(t_u32[:], t_dinv[:], t_state[:])
        t_u16 = pp.tile([128, NPP], f16)
        nc.vector.tensor_copy(t_u16[:], t_u32[:])
        nc.sync.dma_start(u_stage.ap().rearrange("(p n) -> p n", p=128),
                          t_u16[:])
        t_utab = pp.tile([128, SH], f16)
        nc.sync.dma_start(t_utab[:], u_stage.ap().partition_broadcast(128))

        # ---- tail-layout tables (owned-dst order, host-permuted) ---------
        t_deg2 = pp.tile([128, NPP], f32)
        nc.sync.dma_start(t_deg2[:], deg_own.rearrange("(p n) -> p n", p=128))
        t_state2 = pp.tile([128, NPP], f32)
        nc.sync.dma_start(t_state2[:], state_own.rearrange("(p n) -> p n", p=128))
        t_dinv2 = pp.tile([128, NPP], f32)
        nc.vector.reciprocal(t_tmp[:], t_deg2[:])
        nc.scalar.activation(t_dinv2[:], t_tmp[:], AF.Sqrt)
        t_uown = pp.tile([128, NPP], f32)
        nc.vector.tensor_mul(t_uown[:], t_dinv2[:], t_state2[:])

        # ---- structure tensors ------------------------------------------
        t_i1a = pp.tile([128, n1a], i16)
        nc.sync.dma_start(t_i1a[:], i1a_t[:])
        t_i1b = pp.tile([128, n1b], i16)
        nc.sync.dma_start(t_i1b[:], i1b_t[:])
        t_mA = pp.tile([128, W], f16)
        nc.sync.dma_start(t_mA[:], mA_t[:])
        t_iB0 = pp.tile([128, W], i16)
        nc.sync.dma_start(t_iB0[:], iB0_t[:])
        t_iB1 = pp.tile([128, W], i16)
        nc.sync.dma_start(t_iB1[:], iB1_t[:])
        t_mB = pp.tile([128, W], f16)
        nc.sync.dma_start(t_mB[:], mB_t[:])
        t_iC = pp.tile([128, W], i16)
        nc.sync.dma_start(t_iC[:], iC_t[:])

        # ---- 1. scatter run-start values --------------------------------
        t_v0 = pp.tile([128, W], f16)
        nc.gpsimd.local_scatter(
            t_v0[:, 0:W0], t_utab[:, 0:b0], t_i1a[:],
            channels=128, num_elems=W0, num_idxs=n1a)
        nc.gpsimd.local_scatter(
            t_v0[:, W0:W], t_utab[:, a1:SH], t_i1b[:],
            channels=128, num_elems=W - W0, num_idxs=n1b)

        # ---- 2. fill-forward scan: state = mA*state + v0 ----------------
        t_w16 = pp.tile([128, W], f16)
        nc.vector.tensor_tensor_scan(
            t_w16[:], t_mA[:], t_v0[:], 0.0,
            op0=ALU.mult, op1=ALU.add)

        # ---- 3. permute to dst-sorted order -----------------------------
        t_z = pp.tile([128, W], f16)
        nc.gpsimd.local_scatter(
            t_z[:, 0:W0], t_w16[:], t_iB0[:],
            channels=128, num_elems=W0, num_idxs=W)
        nc.gpsimd.local_scatter(
            t_z[:, W0:W], t_w16[:], t_iB1[:],
            channels=128, num_elems=W - W0, num_idxs=W)

        # ---- 4. segment-sum scan: state = mB*state + z ------------------
        t_seg = pp.tile([128, W], f16)
        nc.vector.tensor_tensor_scan(
            t_seg[:], t_mB[:], t_z[:], 0.0,
            op0=ALU.mult, op1=ALU.add)

        # ---- 5. extract segment ends into accumulator -------------------
        t_acc16 = pp.tile([128, OPL], f16)
        nc.gpsimd.local_scatter(
            t_acc16[:], t_seg[:], t_iC[:],
            channels=128, num_elems=OPL, num_idxs=W)
        t_acc32 = pp.tile([128, OPL], f32)
        nc.vector.tensor_copy(t_acc32[:], t_acc16[:])
        nc.sync.dma_start(rs_in.ap().rearrange("(p o) -> p o", p=128),
                          t_acc32[:])

        # ---- 6. ReduceScatter -------------------------------------------
        nc.gpsimd.collective_compute(
            "ReduceScatter", mybir.AluOpType.add,
            ins=[rs_in.ap()[:]], outs=[rs_out.ap()[:]],
            replica_groups=replica,
        )

        # ---- 7. tail -----------------------------------------------------
        t_agg = pp.tile([128, NPP], f32)
        nc.sync.dma_start(t_agg[:], rs_out.ap().rearrange("(p n) -> p n", p=128))
        t_s1 = pp.tile([128, NPP], f32)
        nc.vector.tensor_add(t_s1[:], t_agg[:], t_uown[:])
        nc.vector.tensor_mul(t_s1[:], t_s1[:], t_dinv2[:])

        # stats partials -> ones-matmul -> AllReduce
        NSTAT = 2
        t_pr = small.tile([128, NSTAT], f32)
        t_sq = small.tile([128, NPP], f32)
        nc.vector.tensor_reduce(t_pr[:, 0:1], t_s1[:], axis=mybir.AxisListType.X,
                                op=ALU.add)
        nc.vector.tensor_mul(t_sq[:], t_s1[:], t_s1[:])
        nc.vector.tensor_reduce(t_pr[:, 1:2], t_sq[:], axis=mybir.AxisListType.X,
                                op=ALU.add)
        t_ones = small.tile([128, 1], f32)
        nc.vector.memset(t_ones[:], 1.0)
        ps_st = psum.tile([NSTAT, 1], f32, space="PSUM")
        nc.tensor.matmul(ps_st[:], lhsT=t_pr[:], rhs=t_ones[:], start=True,
                         stop=True)
        t_st = small.tile([NSTAT, 1], f32)
        nc.vector.tensor_copy(t_st[:], ps_st[:])
        nc.sync.dma_start(ar_in.ap()[0:NSTAT], t_st[:].rearrange("p n -> (p n)"))
        t_z8 = small.tile([1, 8 - NSTAT], f32)
        nc.vector.memset(t_z8[:], 0.0)
        nc.sync.dma_start(ar_in.ap()[NSTAT:8], t_z8[:].rearrange("p n -> (p n)"))

        nc.gpsimd.collective_compute(
            "AllReduce", mybir.AluOpType.add,
            ins=[ar_in.ap()[:]], outs=[ar_out.ap()[:]],
            replica_groups=replica,
        )
        t_stats = small.tile([128, 8], f32)
        nc.sync.dma_start(t_stats[:], ar_out.ap().partition_broadcast(128))

        # per-channel coefficients
        t_W = small.tile([128, 1], f32)
        nc.sync.dma_start(t_W[:], gcn_W.rearrange("o h -> h o"))
        t_gam = small.tile([128, 1], f32)
        nc.sync.dma_start(t_gam[:], bn_gamma.rearrange("(h o) -> h o", o=1))
        t_bet = small.tile([128, 1], f32)
        nc.sync.dma_start(t_bet[:], bn_beta.rearrange("(h o) -> h o", o=1))
        t_lW = small.tile([128, OUT], f32)
        nc.sync.dma_start(t_lW[:], lin_W[:])

        inv_n = 1.0 / float(N)
        t_m = small.tile([128, 2], f32)  # m1, c11
        nc.vector.tensor_scalar_mul(t_m[:, 0:1], t_stats[:, 0:1], inv_n)
        nc.vector.tensor_scalar_mul(t_m[:, 1:2], t_stats[:, 1:2], inv_n)
        t_t1 = small.tile([128, 1], f32)
        nc.vector.tensor_mul(t_t1[:], t_m[:, 0:1], t_m[:, 0:1])
        nc.vector.tensor_tensor(t_m[:, 1:2], t_m[:, 1:2], t_t1[:],
                                op=ALU.subtract)

        t_var = small.tile([128, 1], f32)
        t_w2 = small.tile([128, 1], f32)
        nc.vector.tensor_mul(t_w2[:], t_W[:], t_W[:])
        nc.vector.tensor_mul(t_var[:], t_w2[:], t_m[:, 1:2])
        t_isd = small.tile([128, 1], f32)
        t_vpe = small.tile([128, 1], f32)
        nc.vector.tensor_scalar_add(t_vpe[:], t_var[:], BN_EPS)
        nc.vector.reciprocal(t_vpe[:], t_vpe[:])
        nc.scalar.activation(t_isd[:], t_vpe[:], AF.Sqrt)
        t_A = small.tile([128, 1], f32)
        nc.vector.tensor_mul(t_A[:], t_gam[:], t_W[:])
        nc.vector.tensor_mul(t_A[:], t_A[:], t_isd[:])

        # a_o = sum_ch A*linW ; bet_o = sum_ch beta*linW
        ps_c = psum.tile([OUT, 2], f32, space="PSUM")
        nc.tensor.matmul(ps_c[:, 0:1], lhsT=t_lW[:], rhs=t_A[:], start=True,
                         stop=True)
        nc.tensor.matmul(ps_c[:, 1:2], lhsT=t_lW[:], rhs=t_bet[:], start=True,
                         stop=True)
        t_co = small.tile([OUT, 2], f32)
        nc.vector.tensor_copy(t_co[:], ps_c[:])

        # c_o = -m1*a_o + bet_o + lin_b[o]
        coef_stage = nc.dram_tensor("coef_stage", [OUT, 2], f32)
        t_lb = small.tile([OUT, 1], f32)
        nc.sync.dma_start(t_lb[:], lin_b.rearrange("(o k) -> o k", k=1))
        t_cfin = small.tile([OUT, 2], f32)  # [a, c]
        nc.vector.tensor_copy(t_cfin[:, 0:1], t_co[:, 0:1])
        t_ct = small.tile([OUT, 1], f32)
        nc.vector.tensor_mul(t_ct[:], t_co[:, 0:1], t_m[0:OUT, 0:1])
        nc.vector.tensor_tensor(t_cfin[:, 1:2], t_co[:, 1:2], t_ct[:],
                                op=ALU.subtract)
        nc.vector.tensor_add(t_cfin[:, 1:2], t_cfin[:, 1:2], t_lb[:])

        nc.sync.dma_start(coef_stage.ap()[:], t_cfin[:])
        t_coef = small.tile([128, OUT * 2], f32)
        nc.sync.dma_start(
            t_coef[:],
            coef_stage.ap().rearrange("o k -> (o k)").partition_broadcast(128))
        # per partition: [a0, c0, a1, c1]

        # logits + softmax (softmax over 2 = sigmoid of diff)
        t_l = pp.tile([128, NPP, OUT], f32)
        for o in range(OUT):
            nc.vector.tensor_scalar_mul(t_l[:, :, o], t_s1[:],
                                        t_coef[:, 2 * o: 2 * o + 1])
            nc.vector.tensor_scalar(t_l[:, :, o], t_l[:, :, o],
                                    t_coef[:, 2 * o + 1: 2 * o + 2], None,
                                    op0=ALU.add)
            nc.vector.tensor_scalar_max(t_l[:, :, o], t_l[:, :, o], 0.0)

        t_zd = small.tile([128, NPP], f32)
        nc.vector.tensor_tensor(t_zd[:], t_l[:, :, 1], t_l[:, :, 0],
                                op=ALU.subtract)
        t_res = pp.tile([128, NPP, OUT], f32)
        nc.scalar.activation(t_res[:, :, 1], t_zd[:], AF.Sigmoid)
        nc.vector.tensor_scalar(t_res[:, :, 0], t_res[:, :, 1], 1.0, None,
                                op0=ALU.subtract)
        nc.vector.tensor_scalar_mul(t_res[:, :, 0], t_res[:, :, 0], -1.0)

        nc.sync.dma_start(out_t.rearrange("(p n) d -> p n d", p=128), t_res[:])

    nc.compile()
    return nc


_NC_CACHE = {}


def _kernel_s(state, edge_index, gcn_W, gcn_b, bn_gamma, bn_beta, lin_W, lin_b):
    global _LAST_EXEC_NS
    from concourse.bass_utils import run_bass_kernel_spmd

    in_maps, W, b0, a1 = _host_prep_s(state, edge_index)
    key = ("s", W, b0, a1)
    if key not in _NC_CACHE:
        _NC_CACHE[key] = _build_nc_s(W, b0, a1)
    nc = _NC_CACHE[key]

    shared = {
        "gcn_W": np.asarray(gcn_W, dtype=np.float32),
        "bn_gamma": np.asarray(bn_gamma, dtype=np.float32),
        "bn_beta": np.asarray(bn_beta, dtype=np.float32),
        "lin_W": np.asarray(lin_W, dtype=np.float32),
        "lin_b": np.asarray(lin_b, dtype=np.float32),
    }
    for m in in_maps:
        m.update(shared)

    trace = os.environ.get("BASS_GCN_TRACE", "0") == "1"
    res = run_bass_kernel_spmd(nc, in_maps, list(range(NCORES)), trace=trace)
    _LAST_EXEC_NS = res.exec_time_ns

    out = np.empty((N, OUT), dtype=np.float32)
    i_ = np.arange(SH, dtype=np.int64)
    for c in range(NCORES):
        dstg = (i_ % OPL) * NLANE + 16 * c + i_ // OPL
        valid = dstg < N
        out[dstg[valid]] = res.results[c]["out"][valid]
    return out


def kernel(state, edge_index, gcn_W, gcn_b, bn_gamma, bn_beta, lin_W, lin_b):
    global _LAST_EXEC_NS
    if float(np.abs(np.asarray(gcn_b)).max()) == 0.0:
        return _kernel_s(state, edge_index, gcn_W, gcn_b, bn_gamma, bn_beta,
                         lin_W, lin_b)
    # fallback: original implementation (handles gcn_b != 0)
    import kernel_v1_backup as _v1
    out = _v1.kernel(state, edge_index, gcn_W, gcn_b, bn_gamma, bn_beta,
                     lin_W, lin_b)
    _LAST_EXEC_NS = _v1._LAST_EXEC_NS
    return out
